# revision 16
# baseline (speedup 1.0000x reference)
"""BiQRNN forward kernel for Trainium2 (8 NeuronCores, batch-sharded).

Model (see reference):
  ev  = X[:,:,0] (int ids), num = X[:,:,1:]
  e   = emb[ev]                      [B,S,256]
  n   = num @ Wn + bn                [B,S,4]
  c   = [e, n]                       [B,S,260]
  g   = c @ W + b  (W in {Wf,Wb})    -> Z = tanh(g[:,:512]), F = sigmoid(g[:,512:1024])
  hf  = fo_pool(Zf,Ff)[-1]  (h_t = F h_{t-1} + (1-F) Z)
  hb  = (1-Fb[S-1]) * Zb[S-1]        (only last step of reversed scan survives)
  out = [hf, hb] @ Wo + bo           [B,1]

Per core (8 batches), software-pipelined with 1-batch lookahead:
  - 4x indirect row-gather per batch -> e_b [128, 4, 256] bf16 (token t = g*128+p)
  - 8 PE transposes -> eT_b [128 d, 2, 512 tok]; one ACT copy drains PSUM
  - gate GEMM: G^T[h, tok] = matmul(lhsT=W-chunk, rhs=eT-slice); K-passes
    emb[0:128], emb[128:256], then [num(7)+ones(1)] in 4 concurrent PE
    row-strips (Wn/bias folded on host)
  - fo-pool via u-substitution: u_t = (d_t + u_{t-1}) * f_t with
    d_t = z_{t-1} - z_t, h = u + z. The stt (s-1)*z disappears; d is one
    shifted tensor_tensor subtract at DVE 2x rate. tanh writes z shifted
    one column right of sigmoid's f so d + reset columns fall out of one
    flat subtract; two independent scans (chunks 0-1 / 2-3) per batch.
  - backward direction transposed: gb[b,h] via lhsT=eTlast -> 6 large-N
    matmuls; projection via scalar_tensor_tensor accum_out
  - output projection: 4 small fp32 matmuls + bias matmul + add
"""
import numpy as np

import concourse.bacc as bacc
import concourse.bass as bass
import concourse.mybir as mybir
import concourse.tile as tile
from concourse import bass_utils

F32 = mybir.dt.float32
BF16 = mybir.dt.bfloat16
I32 = mybir.dt.int32
NP_BF16 = mybir.dt.np(BF16)

VOCAB, EMB, HID, OUT = 1000, 256, 512, 1
NUM_IN, NUM_OUT = 7, 4
B, S = 64, 512
NCORES = 8
BC = B // NCORES          # 8 batches per core
NT = BC * S               # 4096 tokens per core
SR = S + 1                # per-chunk scan segment (with boundary column)
AF = mybir.ActivationFunctionType
ALU = mybir.AluOpType

N_WARMUP_MM = 28
NOH = 1


def build_kernel(debug=False):
    nc = bacc.Bacc("TRN2", target_bir_lowering=False, debug=debug)

    idx_d = nc.dram_tensor("idx32", [128, 4 * BC + 1], I32, kind="ExternalInput")
    numt1_d = nc.dram_tensor("numt1", [128, NT], BF16, kind="ExternalInput")
    emb_d = nc.dram_tensor("emb", [VOCAB, EMB], BF16, kind="ExternalInput")
    wf_d = nc.dram_tensor("wf", [128, 2 * 2 * HID], BF16, kind="ExternalInput")
    wnfb_d = nc.dram_tensor("wnfb", [128, 2 * HID], BF16, kind="ExternalInput")
    wb_d = nc.dram_tensor("wb", [128, 2 * 2 * HID], BF16, kind="ExternalInput")
    wnbb_d = nc.dram_tensor("wnbb", [128, 2 * HID], BF16, kind="ExternalInput")
    wo_d = nc.dram_tensor("wo", [128, 8], F32, kind="ExternalInput")
    bo_d = nc.dram_tensor("bo", [1, 1], BF16, kind="ExternalInput")
    ident_d = nc.dram_tensor("ident", [128, 128], BF16, kind="ExternalInput")
    FP8 = mybir.dt.float8e4
    embsb_d = nc.dram_tensor("embsb", [128, 8, EMB], BF16, kind="ExternalInput")
    oht0_d = nc.dram_tensor("oht0", [128, 8, S], FP8, kind="ExternalInput")
    oht1_d = nc.dram_tensor("oht1", [128, 8, S], FP8, kind="ExternalInput")
    out_d = nc.dram_tensor("out", [BC, 1], F32, kind="ExternalOutput")

    with tile.TileContext(nc) as tc:
        with tc.tile_pool(name="const", bufs=1) as cpool, \
             tc.tile_pool(name="work", bufs=3) as wpool, \
             tc.tile_pool(name="gath", bufs=8) as gpool, \
             tc.tile_pool(name="ps", bufs=3, space="PSUM") as ps, \
             tc.tile_pool(name="pst", bufs=2, space="PSUM") as pst:
            # ---- constant loads (order matters: gather/transpose deps first) ----
            idx_sb = cpool.tile([128, 4 * BC + 1], I32)
            nc.sync.dma_start(out=idx_sb[:], in_=idx_d[:])
            oht0_sb = cpool.tile([128, 8, S], mybir.dt.float8e4)
            nc.sync.dma_start(out=oht0_sb[:], in_=oht0_d[:])
            embsb_sb = cpool.tile([128, 8, EMB], BF16)
            nc.sync.dma_start(out=embsb_sb[:], in_=embsb_d[:])
            wf_sb = cpool.tile([128, 2048], BF16)
            nc.sync.dma_start(out=wf_sb[:], in_=wf_d[:])
            oht1_sb = cpool.tile([128, 8, S], mybir.dt.float8e4)
            nc.sync.dma_start(out=oht1_sb[:], in_=oht1_d[:])
            wnfb_sb = cpool.tile([128, 1024], BF16)
            nc.sync.dma_start(out=wnfb_sb[:], in_=wnfb_d[:])
            ident = cpool.tile([128, 128], BF16)
            nc.sync.dma_start(out=ident[:], in_=ident_d[:])
            numt1_sb = cpool.tile([128, NT], BF16)
            nc.sync.dma_start(out=numt1_sb[:], in_=numt1_d[:])
            wb_sb = cpool.tile([128, 2048], BF16)
            nc.sync.dma_start(out=wb_sb[:], in_=wb_d[:])
            wnbb_sb = cpool.tile([128, 1024], BF16)
            nc.sync.dma_start(out=wnbb_sb[:], in_=wnbb_d[:])
            wo_sb = cpool.tile([128, 8], F32)
            nc.sync.dma_start(out=wo_sb[:], in_=wo_d[:])
            bo_sb = cpool.tile([1, 1], BF16)
            nc.sync.dma_start(out=bo_sb[:], in_=bo_d[:])

            # ---- PE warmup: open the HAM clock-gate before real work ----
            warm_src = cpool.tile([128, 256], BF16)
            nc.vector.memset(warm_src[:], 0.0)
            wps = ps.tile([128, 2, S], F32, tag="g")
            for _ in range(N_WARMUP_MM):
                nc.tensor.matmul(wps[:, 0, 0:256], lhsT=warm_src[:, 0:128],
                                 rhs=warm_src[:], start=True, stop=True)

            hS = cpool.tile([128, 4, BC], F32)      # forward final states
            eTlast = cpool.tile([128, 2, BC], BF16)  # e^T at t=S-1, per batch
            # persistent Z/F tiles (parity double-buffer); boundary zero
            # columns are written once and never touched by activations
            ZP = [cpool.tile([128, 4, SR], BF16, name=f"ZP{i}") for i in range(2)]
            FP = [cpool.tile([128, 4, SR], BF16, name=f"FP{i}") for i in range(2)]
            for t in ZP:
                nc.vector.memset(t[:, :, 0], 0.0)
            for t in FP:
                nc.vector.memset(t[:, :, S], 0.0)

            def onehot_eT(b):
                # eT[d, k, t] = sum_v emb[v, k*128+d] * onehot[v, t]
                op = ps.tile([128, 2, S], F32, tag="g")
                for k in range(2):
                    for vp in range(8):
                        nc.tensor.matmul(
                            op[:, k, :],
                            lhsT=embsb_sb[:, vp, k * 128:(k + 1) * 128],
                            rhs=(oht0_sb, oht1_sb)[b][:, vp, :],
                            start=(vp == 0), stop=(vp == 7))
                eT_b = wpool.tile([128, 2, S], BF16, tag="eT")
                for k in range(2):
                    nc.scalar.copy(out=eT_b[:, k, :], in_=op[:, k, :])
                return eT_b

            def gather(b):
                e_b = gpool.tile([128, 4, EMB], BF16, tag="eg")
                for g in range(4):
                    nc.gpsimd.indirect_dma_start(
                        out=e_b[:, g, :],
                        out_offset=None,
                        in_=emb_d[:],
                        in_offset=bass.IndirectOffsetOnAxis(
                            ap=idx_sb[:, b * 4 + g:b * 4 + g + 1], axis=0),
                    )
                return e_b

            def transpose(b, e_b):
                tp = pst.tile([128, 2, 4, 128], BF16, tag="tp")
                for g in range(4):
                    for k in range(2):
                        nc.tensor.transpose(
                            out=tp[:, k, g, :],
                            in_=e_b[:, g, k * 128:(k + 1) * 128],
                            identity=ident[:])
                eT_b = wpool.tile([128, 2, S], BF16, tag="eT")
                nc.scalar.copy(
                    out=eT_b[:].rearrange("p a b -> p (a b)"),
                    in_=tp[:].rearrange("p a b c -> p (a b c)"))
                return eT_b

            def gates(b, eT_b):
                """mm12 + strip-parallel mm3p into 4 PSUM tiles."""
                tok = slice(b * S, (b + 1) * S)
                rhs_e0 = eT_b[:, 0, :]
                rhs_e1 = eT_b[:, 1, :]
                zA = ps.tile([128, 2, S], F32, tag="g")
                zB = ps.tile([128, 2, S], F32, tag="g")
                fA = ps.tile([128, 2, S], F32, tag="g")
                fB = ps.tile([128, 2, S], F32, tag="g")
                for half, (tA, tB) in ((0, (zA, zB)), (1, (fA, fB))):
                    off = half * HID
                    for j in range(4):
                        t = (tA, tB)[j // 2][:, j % 2, :]
                        nc.tensor.matmul(t, lhsT=wf_sb[:, off + j * 128:off + j * 128 + 128],
                                         rhs=rhs_e0, start=True, stop=False)
                    for j in range(4):
                        t = (tA, tB)[j // 2][:, j % 2, :]
                        col = 1024 + off + j * 128
                        nc.tensor.matmul(t, lhsT=wf_sb[:, col:col + 128],
                                         rhs=rhs_e1, start=False, stop=False)
                    for j in range(4):
                        t = (tA, tB)[j // 2][:, j % 2, :]
                        col = off + j * 128
                        kw = {}
                        if j > 0:
                            kw = dict(tile_position=(32 * j, 0),
                                      skip_group_check=True)
                        nc.tensor.matmul(
                            t,
                            lhsT=wnfb_sb[32 * j:32 * j + NUM_IN + 1,
                                         col:col + 128],
                            rhs=numt1_sb[32 * j:32 * j + NUM_IN + 1, tok],
                            start=False, stop=True, **kw)
                return zA, zB, fA, fB

            def elementwise(b, zA, zB, fA, fB):
                # Z3[:, j, 1+t] = tanh(gz);  F3[:, j, t] = sigmoid(gf)
                # flat: position c = 513j + t; z leads by one col so
                # d[c] = Z[c] - Z[c+1] = z_{t-1} - z_t with boundary zeros.
                Z3 = ZP[b % 2]
                F3 = FP[b % 2]
                D = wpool.tile([128, 4 * SR - 1], BF16, tag="d")
                U = wpool.tile([128, 4 * SR - 1], F32, tag="u")
                Zf = Z3[:].rearrange("p a b -> p (a b)")
                Ff = F3[:].rearrange("p a b -> p (a b)")
                HSR = 2 * SR  # 1026: flat length of one half (2 chunks)
                nc.scalar.activation(Z3[:, 0:2, 1:SR], zA[:], AF.Tanh)
                nc.scalar.activation(F3[:, 0:2, 0:S], fA[:], AF.Sigmoid)
                nc.vector.tensor_tensor(
                    out=D[:, 0:HSR - 1], in0=Zf[:, 0:HSR - 1],
                    in1=Zf[:, 1:HSR], op=ALU.subtract)
                nc.vector.tensor_tensor_scan(
                    out=U[:, 0:HSR - 1], data0=D[:, 0:HSR - 1],
                    data1=Ff[:, 0:HSR - 1],
                    initial=0.0, op0=ALU.add, op1=ALU.mult)
                nc.scalar.activation(Z3[:, 2:4, 1:SR], zB[:], AF.Tanh)
                nc.scalar.activation(F3[:, 2:4, 0:S], fB[:], AF.Sigmoid)
                nc.vector.tensor_tensor(
                    out=D[:, HSR:2 * HSR - 1], in0=Zf[:, HSR:2 * HSR - 1],
                    in1=Zf[:, HSR + 1:2 * HSR], op=ALU.subtract)
                nc.vector.tensor_tensor_scan(
                    out=U[:, HSR:2 * HSR - 1], data0=D[:, HSR:2 * HSR - 1],
                    data1=Ff[:, HSR:2 * HSR - 1],
                    initial=0.0, op0=ALU.add, op1=ALU.mult)
                # h_{j,S-1} = u + z at the chunk tails; split for the last
                # batch so the projection can start after scanA
                if b == BC - 1:
                    nc.vector.tensor_tensor(
                        out=hS[:, 0:2, b], in0=U[:, S - 1:HSR:SR],
                        in1=Z3[:, 0:2, S], op=ALU.add)
                    nc.vector.tensor_tensor(
                        out=hS[:, 2:4, b], in0=U[:, HSR + S - 1::SR],
                        in1=Z3[:, 2:4, S], op=ALU.add)
                else:
                    nc.vector.tensor_tensor(
                        out=hS[:, :, b], in0=U[:, S - 1::SR], in1=Z3[:, :, S],
                        op=ALU.add)

            # ---- pipelined forward over batches (lookahead 1) ----
            # batches 0..NOH-1 build eT via one-hot matmuls (no gather, PE
            # busy from t~=10us); the rest use indirect gathers + PE transposes
            eTq = [onehot_eT(b) for b in range(NOH)]
            e_next = gather(NOH) if NOH < BC else None
            for b in range(BC):
                if b + 1 >= NOH and b + 1 < BC:
                    eT_next_gather = e_next
                    e_next = gather(b + 2) if b + 2 < BC else None
                zA, zB, fA, fB = gates(b, eTq[b])
                if b + 1 >= NOH and b + 1 < BC:
                    eTq.append(transpose(b + 1, eT_next_gather))
                elementwise(b, zA, zB, fA, fB)

            # ---- backward direction (only t = S-1 matters), [h, b] form ----
            # eTlast: gather last-token emb rows, 2 PE transposes, ACT drain
            e_last = cpool.tile([128, EMB], BF16)
            nc.gpsimd.indirect_dma_start(
                out=e_last[:], out_offset=None, in_=emb_d[:],
                in_offset=bass.IndirectOffsetOnAxis(
                    ap=idx_sb[:, 4 * BC:4 * BC + 1], axis=0))
            lps = pst.tile([128, 2, BC], BF16, tag="tp")
            for k in range(2):
                nc.tensor.transpose(out=lps[:, k, :],
                                    in_=e_last[0:BC, k * 128:(k + 1) * 128],
                                    identity=ident[0:BC, 0:BC])
            nc.scalar.copy(out=eTlast[:], in_=lps[:])
            rhs_e0 = eTlast[:, 0, :]
            rhs_e1 = eTlast[:, 1, :]
            nlast = numt1_sb[0:NUM_IN + 1, S - 1::S]   # [8, BC]
            zbps = pst.tile([128, 4, BC], F32, tag="tp")
            fbps = pst.tile([128, 4, BC], F32, tag="tp")
            for tps, off in ((zbps, 0), (fbps, HID)):
                for j in range(4):
                    col = off + j * 128
                    nc.tensor.matmul(tps[:, j, :], lhsT=wb_sb[:, col:col + 128],
                                     rhs=rhs_e0, start=True, stop=False)
                    nc.tensor.matmul(tps[:, j, :],
                                     lhsT=wb_sb[:, 1024 + col:1024 + col + 128],
                                     rhs=rhs_e1, start=False, stop=False)
                    nc.tensor.matmul(tps[:, j, :],
                                     lhsT=wnbb_sb[0:NUM_IN + 1, col:col + 128],
                                     rhs=nlast, start=False, stop=True)
            zb_t = cpool.tile([128, 4, BC], BF16)
            sb_t = cpool.tile([128, 4, BC], BF16)
            nc.scalar.activation(zb_t[:], zbps[:], AF.Tanh)
            nc.scalar.activation(sb_t[:], fbps[:], AF.Sigmoid)
            wtb = cpool.tile([128, 4, BC], F32)
            nc.vector.scalar_tensor_tensor(
                out=wtb[:], in0=sb_t[:], scalar=1.0, in1=zb_t[:],
                op0=ALU.subtract, op1=ALU.mult)

            # ---- output projection ----
            # out[b] = sum_j hS[:,j,b].Wo_j - wtb[:,j,b].Wo_bj + bo
            # (wo columns 4..7 hold NEGATED backward Wo chunks)
            ops = ps.tile([BC, 1], F32, tag="g")
            for j in range(4):
                nc.tensor.matmul(ops[:], lhsT=hS[:, j, :], rhs=wo_sb[:, j:j + 1],
                                 start=(j == 0), stop=False)
            for j in range(4):
                nc.tensor.matmul(ops[:], lhsT=wtb[:, j, :], rhs=wo_sb[:, 4 + j:5 + j],
                                 start=False, stop=False)
            ones_sb = cpool.tile([1, BC], BF16)
            nc.vector.memset(ones_sb[:], 1.0)
            nc.tensor.matmul(ops[:], lhsT=ones_sb[:],
                             rhs=bo_sb[:], start=False, stop=True)
            out_sb = cpool.tile([BC, 1], F32)
            nc.vector.tensor_copy(out=out_sb[:], in_=ops[:])
            nc.sync.dma_start(out=out_d[:], in_=out_sb[:])

    nc.compile()
    return nc


def prep_inputs(X, emb, Wn, bn, Wf, bf, Wb, bb, Wo, bo):
    """Host-side sharding + weight folding. Returns per-core input maps."""
    X = np.asarray(X, np.float32)
    emb = np.asarray(emb, np.float32)
    Wn = np.asarray(Wn, np.float32)
    bn = np.asarray(bn, np.float32)
    Wf = np.asarray(Wf, np.float32)
    bf_ = np.asarray(bf, np.float32)
    Wb = np.asarray(Wb, np.float32)
    bb_ = np.asarray(bb, np.float32)
    Wo = np.asarray(Wo, np.float32)
    bo_ = np.asarray(bo, np.float32)

    ev = X[:, :, 0].astype(np.int32)                       # [B,S]
    num = X[:, :, 1:]                                      # [B,S,7]

    def fold(W, bvec):
        Wzf = W[:, :2 * HID]                               # drop unused O gate
        w_emb = Wzf[:EMB]                                  # [256,1024]
        wf_resh = w_emb.reshape(2, 128, 2 * HID).transpose(1, 0, 2).reshape(128, 2 * 2 * HID)
        wnf = Wn @ Wzf[EMB:]                               # [7,1024]
        bias_eff = bvec[:2 * HID] + bn @ Wzf[EMB:]         # [1024]
        wnfb = np.concatenate([wnf, bias_eff[None, :]], axis=0)  # [8,1024]
        wnfb_rep = np.zeros((128, 2 * HID), np.float32)
        for strip in range(4):
            wnfb_rep[32 * strip:32 * strip + NUM_IN + 1] = wnfb
        return wf_resh.astype(NP_BF16), wnfb_rep.astype(NP_BF16)

    wf_resh, wnfb = fold(Wf, bf_)
    wb_resh, wnbb = fold(Wb, bb_)

    wo_resh = np.empty((128, 8), np.float32)
    for j in range(4):
        wo_resh[:, j] = Wo[j * 128:(j + 1) * 128, 0]
        wo_resh[:, 4 + j] = -Wo[HID + j * 128:HID + (j + 1) * 128, 0]

    emb_bf = emb.astype(NP_BF16)
    bo_bf = bo_.reshape(1, 1).astype(NP_BF16)
    ident = np.eye(128, dtype=np.float32).astype(NP_BF16)

    in_maps = []
    for c in range(NCORES):
        bs = slice(c * BC, (c + 1) * BC)
        # token t = g*128 + p of local batch b sits at idx32[p, b*4 + g]
        ev_core = ev[bs]                                    # [BC, S]
        idx_wrapped = np.ascontiguousarray(
            ev_core.reshape(BC, 4, 128).transpose(2, 0, 1).reshape(128, 4 * BC))
        idxlast = np.zeros((128, 1), np.int32)
        idxlast[:BC, 0] = ev_core[:, S - 1]
        idx_wrapped = np.concatenate([idx_wrapped, idxlast], axis=1)
        numt = num[bs].transpose(2, 0, 1).reshape(NUM_IN, NT)
        numt1 = np.zeros((128, NT), np.float32)
        for strip in range(4):
            numt1[32 * strip:32 * strip + NUM_IN] = numt
            numt1[32 * strip + NUM_IN] = 1.0
        # one-hot (fp8) + emb-in-sbuf layout for the PE embedding path
        NP_FP8 = mybir.dt.np(mybir.dt.float8e4)
        embsb = np.zeros((128, 8, EMB), np.float32)
        for vp in range(8):
            nrows = min(128, VOCAB - vp * 128)
            if nrows > 0:
                embsb[:nrows, vp] = emb[vp * 128:vp * 128 + nrows]
        oht = np.zeros((2, 128, 8, S), np.float32)
        for bi in range(2):
            evb = ev_core[bi]
            oht[bi, evb % 128, evb // 128, np.arange(S)] = 1.0
        in_maps.append({
            "idx32": idx_wrapped,
            "embsb": embsb.astype(NP_BF16),
            "oht0": oht[0].astype(NP_FP8),
            "oht1": oht[1].astype(NP_FP8),
            "numt1": numt1.astype(NP_BF16),
            "emb": emb_bf,
            "ident": ident,
            "wf": wf_resh, "wnfb": wnfb,
            "wb": wb_resh, "wnbb": wnbb,
            "wo": wo_resh, "bo": bo_bf,
        })
    return in_maps


_NC_CACHE = {}


def kernel(X, emb, Wn, bn, Wf, bf, Wb, bb, Wo, bo):
    if "nc" not in _NC_CACHE:
        _NC_CACHE["nc"] = build_kernel()
    nc = _NC_CACHE["nc"]
    in_maps = prep_inputs(X, emb, Wn, bn, Wf, bf, Wb, bb, Wo, bo)
    res = bass_utils.run_bass_kernel_spmd(nc, in_maps, core_ids=list(range(NCORES)))
    return np.concatenate([res.results[c]["out"] for c in range(NCORES)], axis=0)


# revision 17
# speedup vs baseline: 1.0916x; 1.0916x over previous
"""BiQRNN forward kernel for Trainium2 (8 NeuronCores, batch-sharded).

Model (see reference):
  ev  = X[:,:,0] (int ids), num = X[:,:,1:]
  e   = emb[ev]                      [B,S,256]
  n   = num @ Wn + bn                [B,S,4]
  c   = [e, n]                       [B,S,260]
  g   = c @ W + b  (W in {Wf,Wb})    -> Z = tanh(g[:,:512]), F = sigmoid(g[:,512:1024])
  hf  = fo_pool(Zf,Ff)[-1]  (h_t = F h_{t-1} + (1-F) Z)
  hb  = (1-Fb[S-1]) * Zb[S-1]        (only last step of reversed scan survives)
  out = [hf, hb] @ Wo + bo           [B,1]

Per core (8 batches), software-pipelined with 1-batch lookahead:
  - 4x indirect row-gather per batch -> e_b [128, 4, 256] bf16 (token t = g*128+p)
  - 8 PE transposes -> eT_b [128 d, 2, 512 tok]; one ACT copy drains PSUM
  - gate GEMM: G^T[h, tok] = matmul(lhsT=W-chunk, rhs=eT-slice); K-passes
    emb[0:128], emb[128:256], then [num(7)+ones(1)] in 4 concurrent PE
    row-strips (Wn/bias folded on host)
  - fo-pool via u-substitution: u_t = (d_t + u_{t-1}) * f_t with
    d_t = z_{t-1} - z_t, h = u + z. The stt (s-1)*z disappears; d is one
    shifted tensor_tensor subtract at DVE 2x rate. tanh writes z shifted
    one column right of sigmoid's f so d + reset columns fall out of one
    flat subtract; two independent scans (chunks 0-1 / 2-3) per batch.
  - backward direction transposed: gb[b,h] via lhsT=eTlast -> 6 large-N
    matmuls; projection via scalar_tensor_tensor accum_out
  - output projection: 4 small fp32 matmuls + bias matmul + add
"""
import numpy as np

import concourse.bacc as bacc
import concourse.bass as bass
import concourse.mybir as mybir
import concourse.tile as tile
from concourse import bass_utils

F32 = mybir.dt.float32
BF16 = mybir.dt.bfloat16
I32 = mybir.dt.int32
NP_BF16 = mybir.dt.np(BF16)

VOCAB, EMB, HID, OUT = 1000, 256, 512, 1
NUM_IN, NUM_OUT = 7, 4
B, S = 64, 512
NCORES = 8
BC = B // NCORES          # 8 batches per core
NT = BC * S               # 4096 tokens per core
SR = S + 1                # per-chunk scan segment (with boundary column)
AF = mybir.ActivationFunctionType
ALU = mybir.AluOpType

N_WARMUP_MM = 28
NOH = 2


def build_kernel(debug=False):
    nc = bacc.Bacc("TRN2", target_bir_lowering=False, debug=debug)

    idx_d = nc.dram_tensor("idx32", [128, 4 * BC + 1], I32, kind="ExternalInput")
    numt1_d = nc.dram_tensor("numt1", [128, NT], BF16, kind="ExternalInput")
    emb_d = nc.dram_tensor("emb", [VOCAB, EMB], BF16, kind="ExternalInput")
    wf_d = nc.dram_tensor("wf", [128, 2 * 2 * HID], BF16, kind="ExternalInput")
    wnfb_d = nc.dram_tensor("wnfb", [128, 2 * HID], BF16, kind="ExternalInput")
    wb_d = nc.dram_tensor("wb", [128, 2 * 2 * HID], BF16, kind="ExternalInput")
    wnbb_d = nc.dram_tensor("wnbb", [128, 2 * HID], BF16, kind="ExternalInput")
    wo_d = nc.dram_tensor("wo", [128, 8], F32, kind="ExternalInput")
    bo_d = nc.dram_tensor("bo", [1, 1], BF16, kind="ExternalInput")
    ident_d = nc.dram_tensor("ident", [128, 128], BF16, kind="ExternalInput")
    FP8 = mybir.dt.float8e4
    embsb_d = nc.dram_tensor("embsb", [128, 8, EMB], BF16, kind="ExternalInput")
    oht0_d = nc.dram_tensor("oht0", [128, 8, S], FP8, kind="ExternalInput")
    oht1_d = nc.dram_tensor("oht1", [128, 8, S], FP8, kind="ExternalInput")
    out_d = nc.dram_tensor("out", [BC, 1], F32, kind="ExternalOutput")

    with tile.TileContext(nc) as tc:
        with tc.tile_pool(name="const", bufs=1) as cpool, \
             tc.tile_pool(name="work", bufs=3) as wpool, \
             tc.tile_pool(name="gath", bufs=8) as gpool, \
             tc.tile_pool(name="ps", bufs=3, space="PSUM") as ps, \
             tc.tile_pool(name="pst", bufs=2, space="PSUM") as pst:
            # ---- constant loads (order matters: gather/transpose deps first) ----
            idx_sb = cpool.tile([128, 4 * BC + 1], I32)
            nc.sync.dma_start(out=idx_sb[:], in_=idx_d[:])
            oht0_sb = cpool.tile([128, 8, S], mybir.dt.float8e4)
            nc.sync.dma_start(out=oht0_sb[:], in_=oht0_d[:])
            embsb_sb = cpool.tile([128, 8, EMB], BF16)
            nc.sync.dma_start(out=embsb_sb[:], in_=embsb_d[:])
            wf_sb = cpool.tile([128, 2048], BF16)
            nc.sync.dma_start(out=wf_sb[:], in_=wf_d[:])
            oht1_sb = cpool.tile([128, 8, S], mybir.dt.float8e4)
            nc.sync.dma_start(out=oht1_sb[:], in_=oht1_d[:])
            wnfb_sb = cpool.tile([128, 1024], BF16)
            nc.sync.dma_start(out=wnfb_sb[:], in_=wnfb_d[:])
            ident = cpool.tile([128, 128], BF16)
            nc.sync.dma_start(out=ident[:], in_=ident_d[:])
            numt1_sb = cpool.tile([128, NT], BF16)
            nc.sync.dma_start(out=numt1_sb[:], in_=numt1_d[:])
            wb_sb = cpool.tile([128, 2048], BF16)
            nc.sync.dma_start(out=wb_sb[:], in_=wb_d[:])
            wnbb_sb = cpool.tile([128, 1024], BF16)
            nc.sync.dma_start(out=wnbb_sb[:], in_=wnbb_d[:])
            wo_sb = cpool.tile([128, 8], F32)
            nc.sync.dma_start(out=wo_sb[:], in_=wo_d[:])
            bo_sb = cpool.tile([1, 1], BF16)
            nc.sync.dma_start(out=bo_sb[:], in_=bo_d[:])

            # ---- PE warmup: open the HAM clock-gate before real work ----
            warm_src = cpool.tile([128, 256], BF16)
            nc.vector.memset(warm_src[:], 0.0)
            wps = ps.tile([128, 2, S], F32, tag="g")
            for _ in range(N_WARMUP_MM):
                nc.tensor.matmul(wps[:, 0, 0:256], lhsT=warm_src[:, 0:128],
                                 rhs=warm_src[:], start=True, stop=True)

            hS = cpool.tile([128, 4, BC], F32)      # forward final states
            eTlast = cpool.tile([128, 2, BC], BF16)  # e^T at t=S-1, per batch
            # persistent Z/F tiles (parity double-buffer); boundary zero
            # columns are written once and never touched by activations
            ZP = [cpool.tile([128, 4, SR], BF16, name=f"ZP{i}") for i in range(2)]
            FP = [cpool.tile([128, 4, SR], BF16, name=f"FP{i}") for i in range(2)]
            for t in ZP:
                nc.vector.memset(t[:, :, 0], 0.0)
            for t in FP:
                nc.vector.memset(t[:, :, S], 0.0)

            def onehot_eT(b):
                # eT[d, k, t] = sum_v emb[v, k*128+d] * onehot[v, t]
                op = ps.tile([128, 2, S], F32, tag="g")
                for k in range(2):
                    for vp in range(8):
                        nc.tensor.matmul(
                            op[:, k, :],
                            lhsT=embsb_sb[:, vp, k * 128:(k + 1) * 128],
                            rhs=(oht0_sb, oht1_sb)[b][:, vp, :],
                            start=(vp == 0), stop=(vp == 7))
                eT_b = wpool.tile([128, 2, S], BF16, tag="eT")
                for k in range(2):
                    nc.scalar.copy(out=eT_b[:, k, :], in_=op[:, k, :])
                return eT_b

            def gather(b):
                e_b = gpool.tile([128, 4, EMB], BF16, tag="eg")
                for g in range(4):
                    nc.gpsimd.indirect_dma_start(
                        out=e_b[:, g, :],
                        out_offset=None,
                        in_=emb_d[:],
                        in_offset=bass.IndirectOffsetOnAxis(
                            ap=idx_sb[:, b * 4 + g:b * 4 + g + 1], axis=0),
                    )
                return e_b

            def transpose(b, e_b):
                tp = pst.tile([128, 2, 4, 128], BF16, tag="tp")
                for g in range(4):
                    for k in range(2):
                        nc.tensor.transpose(
                            out=tp[:, k, g, :],
                            in_=e_b[:, g, k * 128:(k + 1) * 128],
                            identity=ident[:])
                eT_b = wpool.tile([128, 2, S], BF16, tag="eT")
                nc.scalar.copy(
                    out=eT_b[:].rearrange("p a b -> p (a b)"),
                    in_=tp[:].rearrange("p a b c -> p (a b c)"))
                return eT_b

            def gates(b, eT_b):
                """mm12 + strip-parallel mm3p into 4 PSUM tiles."""
                tok = slice(b * S, (b + 1) * S)
                rhs_e0 = eT_b[:, 0, :]
                rhs_e1 = eT_b[:, 1, :]
                zA = ps.tile([128, 2, S], F32, tag="g")
                zB = ps.tile([128, 2, S], F32, tag="g")
                fA = ps.tile([128, 2, S], F32, tag="g")
                fB = ps.tile([128, 2, S], F32, tag="g")
                for half, (tA, tB) in ((0, (zA, zB)), (1, (fA, fB))):
                    off = half * HID
                    for j in range(4):
                        t = (tA, tB)[j // 2][:, j % 2, :]
                        nc.tensor.matmul(t, lhsT=wf_sb[:, off + j * 128:off + j * 128 + 128],
                                         rhs=rhs_e0, start=True, stop=False)
                    for j in range(4):
                        t = (tA, tB)[j // 2][:, j % 2, :]
                        col = 1024 + off + j * 128
                        nc.tensor.matmul(t, lhsT=wf_sb[:, col:col + 128],
                                         rhs=rhs_e1, start=False, stop=False)
                    for j in range(4):
                        t = (tA, tB)[j // 2][:, j % 2, :]
                        col = off + j * 128
                        kw = {}
                        if j > 0:
                            kw = dict(tile_position=(32 * j, 0),
                                      skip_group_check=True)
                        nc.tensor.matmul(
                            t,
                            lhsT=wnfb_sb[32 * j:32 * j + NUM_IN + 1,
                                         col:col + 128],
                            rhs=numt1_sb[32 * j:32 * j + NUM_IN + 1, tok],
                            start=False, stop=True, **kw)
                return zA, zB, fA, fB

            def elementwise(b, zA, zB, fA, fB):
                # Z3[:, j, 1+t] = tanh(gz);  F3[:, j, t] = sigmoid(gf)
                # flat: position c = 513j + t; z leads by one col so
                # d[c] = Z[c] - Z[c+1] = z_{t-1} - z_t with boundary zeros.
                Z3 = ZP[b % 2]
                F3 = FP[b % 2]
                D = wpool.tile([128, 4 * SR - 1], BF16, tag="d")
                U = wpool.tile([128, 4 * SR - 1], F32, tag="u")
                Zf = Z3[:].rearrange("p a b -> p (a b)")
                Ff = F3[:].rearrange("p a b -> p (a b)")
                HSR = 2 * SR  # 1026: flat length of one half (2 chunks)
                nc.scalar.activation(Z3[:, 0:2, 1:SR], zA[:], AF.Tanh)
                nc.scalar.activation(F3[:, 0:2, 0:S], fA[:], AF.Sigmoid)
                nc.vector.tensor_tensor(
                    out=D[:, 0:HSR - 1], in0=Zf[:, 0:HSR - 1],
                    in1=Zf[:, 1:HSR], op=ALU.subtract)
                nc.vector.tensor_tensor_scan(
                    out=U[:, 0:HSR - 1], data0=D[:, 0:HSR - 1],
                    data1=Ff[:, 0:HSR - 1],
                    initial=0.0, op0=ALU.add, op1=ALU.mult)
                nc.scalar.activation(Z3[:, 2:4, 1:SR], zB[:], AF.Tanh)
                nc.scalar.activation(F3[:, 2:4, 0:S], fB[:], AF.Sigmoid)
                nc.vector.tensor_tensor(
                    out=D[:, HSR:2 * HSR - 1], in0=Zf[:, HSR:2 * HSR - 1],
                    in1=Zf[:, HSR + 1:2 * HSR], op=ALU.subtract)
                nc.vector.tensor_tensor_scan(
                    out=U[:, HSR:2 * HSR - 1], data0=D[:, HSR:2 * HSR - 1],
                    data1=Ff[:, HSR:2 * HSR - 1],
                    initial=0.0, op0=ALU.add, op1=ALU.mult)
                # h_{j,S-1} = u + z at the chunk tails; split for the last
                # batch so the projection can start after scanA
                if b == BC - 1:
                    nc.vector.tensor_tensor(
                        out=hS[:, 0:2, b], in0=U[:, S - 1:HSR:SR],
                        in1=Z3[:, 0:2, S], op=ALU.add)
                    nc.vector.tensor_tensor(
                        out=hS[:, 2:4, b], in0=U[:, HSR + S - 1::SR],
                        in1=Z3[:, 2:4, S], op=ALU.add)
                else:
                    nc.vector.tensor_tensor(
                        out=hS[:, :, b], in0=U[:, S - 1::SR], in1=Z3[:, :, S],
                        op=ALU.add)

            # ---- pipelined forward over batches (lookahead 1) ----
            # batches 0..NOH-1 build eT via one-hot matmuls (no gather, PE
            # busy from t~=10us); the rest use indirect gathers + PE transposes
            eTq = [onehot_eT(b) for b in range(NOH)]
            e_next = gather(NOH) if NOH < BC else None
            for b in range(BC):
                if b + 1 >= NOH and b + 1 < BC:
                    eT_next_gather = e_next
                    e_next = gather(b + 2) if b + 2 < BC else None
                zA, zB, fA, fB = gates(b, eTq[b])
                if b + 1 >= NOH and b + 1 < BC:
                    eTq.append(transpose(b + 1, eT_next_gather))
                elementwise(b, zA, zB, fA, fB)

            # ---- backward direction (only t = S-1 matters), [h, b] form ----
            # eTlast: gather last-token emb rows, 2 PE transposes, ACT drain
            e_last = cpool.tile([128, EMB], BF16)
            nc.gpsimd.indirect_dma_start(
                out=e_last[:], out_offset=None, in_=emb_d[:],
                in_offset=bass.IndirectOffsetOnAxis(
                    ap=idx_sb[:, 4 * BC:4 * BC + 1], axis=0))
            lps = pst.tile([128, 2, BC], BF16, tag="tp")
            for k in range(2):
                nc.tensor.transpose(out=lps[:, k, :],
                                    in_=e_last[0:BC, k * 128:(k + 1) * 128],
                                    identity=ident[0:BC, 0:BC])
            nc.scalar.copy(out=eTlast[:], in_=lps[:])
            rhs_e0 = eTlast[:, 0, :]
            rhs_e1 = eTlast[:, 1, :]
            nlast = numt1_sb[0:NUM_IN + 1, S - 1::S]   # [8, BC]
            zbps = pst.tile([128, 4, BC], F32, tag="tp")
            fbps = pst.tile([128, 4, BC], F32, tag="tp")
            for tps, off in ((zbps, 0), (fbps, HID)):
                for j in range(4):
                    col = off + j * 128
                    nc.tensor.matmul(tps[:, j, :], lhsT=wb_sb[:, col:col + 128],
                                     rhs=rhs_e0, start=True, stop=False)
                    nc.tensor.matmul(tps[:, j, :],
                                     lhsT=wb_sb[:, 1024 + col:1024 + col + 128],
                                     rhs=rhs_e1, start=False, stop=False)
                    nc.tensor.matmul(tps[:, j, :],
                                     lhsT=wnbb_sb[0:NUM_IN + 1, col:col + 128],
                                     rhs=nlast, start=False, stop=True)
            zb_t = cpool.tile([128, 4, BC], BF16)
            sb_t = cpool.tile([128, 4, BC], BF16)
            nc.scalar.activation(zb_t[:], zbps[:], AF.Tanh)
            nc.scalar.activation(sb_t[:], fbps[:], AF.Sigmoid)
            wtb = cpool.tile([128, 4, BC], F32)
            nc.vector.scalar_tensor_tensor(
                out=wtb[:], in0=sb_t[:], scalar=1.0, in1=zb_t[:],
                op0=ALU.subtract, op1=ALU.mult)

            # ---- output projection ----
            # out[b] = sum_j hS[:,j,b].Wo_j - wtb[:,j,b].Wo_bj + bo
            # (wo columns 4..7 hold NEGATED backward Wo chunks)
            ops = ps.tile([BC, 1], F32, tag="g")
            for j in range(4):
                nc.tensor.matmul(ops[:], lhsT=hS[:, j, :], rhs=wo_sb[:, j:j + 1],
                                 start=(j == 0), stop=False)
            for j in range(4):
                nc.tensor.matmul(ops[:], lhsT=wtb[:, j, :], rhs=wo_sb[:, 4 + j:5 + j],
                                 start=False, stop=False)
            ones_sb = cpool.tile([1, BC], BF16)
            nc.vector.memset(ones_sb[:], 1.0)
            nc.tensor.matmul(ops[:], lhsT=ones_sb[:],
                             rhs=bo_sb[:], start=False, stop=True)
            out_sb = cpool.tile([BC, 1], F32)
            nc.vector.tensor_copy(out=out_sb[:], in_=ops[:])
            nc.sync.dma_start(out=out_d[:], in_=out_sb[:])

    nc.compile()
    return nc


def prep_inputs(X, emb, Wn, bn, Wf, bf, Wb, bb, Wo, bo):
    """Host-side sharding + weight folding. Returns per-core input maps."""
    X = np.asarray(X, np.float32)
    emb = np.asarray(emb, np.float32)
    Wn = np.asarray(Wn, np.float32)
    bn = np.asarray(bn, np.float32)
    Wf = np.asarray(Wf, np.float32)
    bf_ = np.asarray(bf, np.float32)
    Wb = np.asarray(Wb, np.float32)
    bb_ = np.asarray(bb, np.float32)
    Wo = np.asarray(Wo, np.float32)
    bo_ = np.asarray(bo, np.float32)

    ev = X[:, :, 0].astype(np.int32)                       # [B,S]
    num = X[:, :, 1:]                                      # [B,S,7]

    def fold(W, bvec):
        Wzf = W[:, :2 * HID]                               # drop unused O gate
        w_emb = Wzf[:EMB]                                  # [256,1024]
        wf_resh = w_emb.reshape(2, 128, 2 * HID).transpose(1, 0, 2).reshape(128, 2 * 2 * HID)
        wnf = Wn @ Wzf[EMB:]                               # [7,1024]
        bias_eff = bvec[:2 * HID] + bn @ Wzf[EMB:]         # [1024]
        wnfb = np.concatenate([wnf, bias_eff[None, :]], axis=0)  # [8,1024]
        wnfb_rep = np.zeros((128, 2 * HID), np.float32)
        for strip in range(4):
            wnfb_rep[32 * strip:32 * strip + NUM_IN + 1] = wnfb
        return wf_resh.astype(NP_BF16), wnfb_rep.astype(NP_BF16)

    wf_resh, wnfb = fold(Wf, bf_)
    wb_resh, wnbb = fold(Wb, bb_)

    wo_resh = np.empty((128, 8), np.float32)
    for j in range(4):
        wo_resh[:, j] = Wo[j * 128:(j + 1) * 128, 0]
        wo_resh[:, 4 + j] = -Wo[HID + j * 128:HID + (j + 1) * 128, 0]

    emb_bf = emb.astype(NP_BF16)
    bo_bf = bo_.reshape(1, 1).astype(NP_BF16)
    ident = np.eye(128, dtype=np.float32).astype(NP_BF16)

    in_maps = []
    for c in range(NCORES):
        bs = slice(c * BC, (c + 1) * BC)
        # token t = g*128 + p of local batch b sits at idx32[p, b*4 + g]
        ev_core = ev[bs]                                    # [BC, S]
        idx_wrapped = np.ascontiguousarray(
            ev_core.reshape(BC, 4, 128).transpose(2, 0, 1).reshape(128, 4 * BC))
        idxlast = np.zeros((128, 1), np.int32)
        idxlast[:BC, 0] = ev_core[:, S - 1]
        idx_wrapped = np.concatenate([idx_wrapped, idxlast], axis=1)
        numt = num[bs].transpose(2, 0, 1).reshape(NUM_IN, NT)
        numt1 = np.zeros((128, NT), np.float32)
        for strip in range(4):
            numt1[32 * strip:32 * strip + NUM_IN] = numt
            numt1[32 * strip + NUM_IN] = 1.0
        # one-hot (fp8) + emb-in-sbuf layout for the PE embedding path
        NP_FP8 = mybir.dt.np(mybir.dt.float8e4)
        embsb = np.zeros((128, 8, EMB), np.float32)
        for vp in range(8):
            nrows = min(128, VOCAB - vp * 128)
            if nrows > 0:
                embsb[:nrows, vp] = emb[vp * 128:vp * 128 + nrows]
        oht = np.zeros((2, 128, 8, S), np.float32)
        for bi in range(2):
            evb = ev_core[bi]
            oht[bi, evb % 128, evb // 128, np.arange(S)] = 1.0
        in_maps.append({
            "idx32": idx_wrapped,
            "embsb": embsb.astype(NP_BF16),
            "oht0": oht[0].astype(NP_FP8),
            "oht1": oht[1].astype(NP_FP8),
            "numt1": numt1.astype(NP_BF16),
            "emb": emb_bf,
            "ident": ident,
            "wf": wf_resh, "wnfb": wnfb,
            "wb": wb_resh, "wnbb": wnbb,
            "wo": wo_resh, "bo": bo_bf,
        })
    return in_maps


_NC_CACHE = {}


def kernel(X, emb, Wn, bn, Wf, bf, Wb, bb, Wo, bo):
    if "nc" not in _NC_CACHE:
        _NC_CACHE["nc"] = build_kernel()
    nc = _NC_CACHE["nc"]
    in_maps = prep_inputs(X, emb, Wn, bn, Wf, bf, Wb, bb, Wo, bo)
    res = bass_utils.run_bass_kernel_spmd(nc, in_maps, core_ids=list(range(NCORES)))
    return np.concatenate([res.results[c]["out"] for c in range(NCORES)], axis=0)
